# revision 7
# baseline (speedup 1.0000x reference)
"""Trainium2 Bass kernel for nn_BDHModel (topk_masking).

Computes, per head h and token l:
    raw = projections[:, tokens, :]                  (gathered on host = sequence sharding)
    thr[h,l] = 20th largest of raw[h,l,:]
    acts = (raw >= thr)
    preds[h,l] = acts[h,l] @ sigma[h].T
    dot[h,l]   = sum(preds[h,l] * acts[h,l+1])
    norm2[h,l] = sum(preds[h,l]^2)
    out = 1 - dot / (sqrt(norm2)*sqrt(20) + 1e-8)    (final scalar math on host)

Distribution: data-parallel over the sequence across 8 NeuronCores. Each core
processes a 1024-token chunk (plus one boundary token) for all 3 heads; sigma
(pre-transposed, bf16) is replicated to every core.
"""

import os
import numpy as np
import ml_dtypes

import concourse.bacc as bacc
import concourse.mybir as mybir
import concourse.bass_utils as bass_utils
from concourse.tile import TileContext
from concourse.masks import make_identity

H, V, D, L = 3, 32000, 2048, 8192
K = 20
NCORES = 8
CHUNK = L // NCORES            # 1024 tokens per core
TILES = CHUNK // 128 + 1       # 9 row-tiles (last holds the boundary token + pad)
ROWS = TILES * 128             # 1152
DB = D // 128                  # 16 blocks of 128 along the neuron axis
P = 128

F32 = mybir.dt.float32
BF16 = mybir.dt.bfloat16

LAST_RESULTS = None            # test.py reads exec_time_ns from here

_NC_CACHE = None


def _build_nc():
    nc = bacc.Bacc("TRN2", target_bir_lowering=False, debug=False)
    raw_ext = nc.dram_tensor("raw", [H, ROWS, D], F32, kind="ExternalInput")
    sigT_ext = nc.dram_tensor("sigT", [H, DB, P, D], BF16, kind="ExternalInput")
    dot_ext = nc.dram_tensor("dot_out", [1, H, CHUNK], F32, kind="ExternalOutput")
    nrm_ext = nc.dram_tensor("nrm_out", [1, H, CHUNK], F32, kind="ExternalOutput")

    with TileContext(nc) as tc:
        _body(nc, tc, raw_ext, sigT_ext, dot_ext, nrm_ext)
    nc.compile()
    return nc


def _body(nc, tc, raw_ext, sigT_ext, dot_ext, nrm_ext):
    with (
        tc.tile_pool(name="consts", bufs=1) as consts,
        tc.tile_pool(name="sig", bufs=1) as sig_pool,
        tc.tile_pool(name="actsT", bufs=1) as actsT_pool,
        tc.tile_pool(name="raw", bufs=3) as raw_pool,
        tc.tile_pool(name="acts", bufs=3) as acts_pool,
        tc.tile_pool(name="mr", bufs=2) as mr_pool,
        tc.tile_pool(name="m8", bufs=6) as m8_pool,
        tc.tile_pool(name="preds", bufs=3) as preds_pool,
        tc.tile_pool(name="prod", bufs=3) as prod_pool,
        tc.tile_pool(name="stage", bufs=1) as stage_pool,
        tc.tile_pool(name="tpsum", bufs=2, space="PSUM") as tpsum_pool,
        tc.tile_pool(name="gpsum", bufs=2, space="PSUM") as gpsum_pool,
        tc.tile_pool(name="rpsum", bufs=1, space="PSUM") as rpsum_pool,
    ):
        ident = consts.tile([P, P], BF16)
        make_identity(nc, ident[:])
        ones = consts.tile([P, 1], BF16)
        nc.vector.memset(ones[:], 1.0)

        dot_sb = stage_pool.tile([1, H, CHUNK], F32, tag="dot_sb")
        nrm_sb = stage_pool.tile([1, H, CHUNK], F32, tag="nrm_sb")

        for h in range(H):
            sigT_sb = sig_pool.tile([P, DB, D], BF16, tag="sigT")
            for db in range(DB):
                nc.sync.dma_start(sigT_sb[:, db, :], sigT_ext[h, db])

            actsT = actsT_pool.tile([P, DB, ROWS], BF16, tag="actsT")

            # --- gather-free stage: topk threshold + mask + transpose ---
            for t in range(TILES):
                raw_t = raw_pool.tile([P, D], F32, tag="raw")
                nc.sync.dma_start(raw_t[:], raw_ext[h, t * P:(t + 1) * P, :])

                m8a = m8_pool.tile([P, 8], F32, tag="m8")
                nc.vector.max(m8a[:], raw_t[:])
                mra = mr_pool.tile([P, D], F32, tag="mr")
                nc.vector.match_replace(mra[:], m8a[:], raw_t[:], -1e30)
                m8b = m8_pool.tile([P, 8], F32, tag="m8")
                nc.vector.max(m8b[:], mra[:])
                mrb = mr_pool.tile([P, D], F32, tag="mr")
                nc.vector.match_replace(mrb[:], m8b[:], mra[:], -1e30)
                m8c = m8_pool.tile([P, 8], F32, tag="m8")
                nc.vector.max(m8c[:], mrb[:])
                # rank 20 = 8 + 8 + 4  ->  index 3 of the third max8
                acts_t = acts_pool.tile([P, D], BF16, tag="acts")
                nc.vector.tensor_scalar(
                    acts_t[:], raw_t[:], m8c[:, 3:4], None, mybir.AluOpType.is_ge
                )
                for db in range(DB):
                    pst = tpsum_pool.tile([P, P], BF16, tag="tp")
                    nc.tensor.transpose(pst[:], acts_t[:, db * P:(db + 1) * P], ident[:])
                    nc.scalar.copy(actsT[:, db, t * P:(t + 1) * P], pst[:])

            # --- per-head GEMM + reductions ---
            for c in range(CHUNK // 512):
                l0 = c * 512
                dot_ps = rpsum_pool.tile([1, 512], F32, tag="dotps")
                nrm_ps = rpsum_pool.tile([1, 512], F32, tag="nrmps")
                for eb in range(DB):
                    pg = gpsum_pool.tile([P, 512], F32, tag="gemm")
                    for db in range(DB):
                        nc.tensor.matmul(
                            pg[:],
                            sigT_sb[:, db, eb * P:(eb + 1) * P],
                            actsT[:, db, l0:l0 + 512],
                            start=(db == 0),
                            stop=(db == DB - 1),
                        )
                    predsT = preds_pool.tile([P, 512], BF16, tag="preds")
                    nc.scalar.copy(predsT[:], pg[:])
                    prod = prod_pool.tile([P, 512], BF16, tag="prod")
                    nc.vector.tensor_tensor(
                        prod[:], predsT[:], actsT[:, eb, l0 + 1:l0 + 513],
                        op=mybir.AluOpType.mult,
                    )
                    prod2 = prod_pool.tile([P, 512], BF16, tag="prod2")
                    nc.vector.tensor_tensor(
                        prod2[:], predsT[:], predsT[:], op=mybir.AluOpType.mult
                    )
                    nc.tensor.matmul(
                        dot_ps[:], ones[:], prod[:],
                        start=(eb == 0), stop=(eb == DB - 1), skip_group_check=True,
                    )
                    nc.tensor.matmul(
                        nrm_ps[:], ones[:], prod2[:],
                        start=(eb == 0), stop=(eb == DB - 1), skip_group_check=True,
                    )
                nc.scalar.copy(dot_sb[:, h, l0:l0 + 512], dot_ps[:])
                nc.scalar.copy(nrm_sb[:, h, l0:l0 + 512], nrm_ps[:])

            if int(os.environ.get("BDH_HEAD_BARRIER", "0")):
                tc.strict_bb_all_engine_barrier()

        nc.sync.dma_start(dot_ext[:, :, :], dot_sb[:, :, :])
        nc.sync.dma_start(nrm_ext[:, :, :], nrm_sb[:, :, :])


def kernel(tokens, projections, sigmas):
    global LAST_RESULTS, _NC_CACHE
    tokens = np.asarray(tokens)
    projections = np.asarray(projections, dtype=np.float32)
    sigmas = np.asarray(sigmas, dtype=np.float32)

    # host-side shard: gather the token rows (this IS the sequence sharding),
    # pre-transpose sigma to (d, e) blocks in bf16.
    raw = projections[:, tokens, :]                      # (H, L, D) f32
    sigT = np.ascontiguousarray(sigmas.transpose(0, 2, 1))   # (H, D_in, D_out)
    sigT = sigT.reshape(H, DB, P, D).astype(ml_dtypes.bfloat16)

    in_maps = []
    for c in range(NCORES):
        lo = c * CHUNK
        hi = min(lo + CHUNK + 1, L)
        chunk = raw[:, lo:hi, :]                          # (H, <=1025, D)
        pad = ROWS - chunk.shape[1]
        chunk = np.concatenate(
            [chunk, np.repeat(chunk[:, -1:, :], pad, axis=1)], axis=1
        )
        in_maps.append({"raw": np.ascontiguousarray(chunk), "sigT": sigT})

    nc = _NC_CACHE
    if nc is None:
        nc = _NC_CACHE = _build_nc()

    res = bass_utils.run_bass_kernel_spmd(nc, in_maps, core_ids=list(range(NCORES)))
    LAST_RESULTS = res

    dots = np.concatenate([r["dot_out"][0] for r in res.results], axis=1)   # (H, 8192)
    nrm2 = np.concatenate([r["nrm_out"][0] for r in res.results], axis=1)
    dots = dots[:, : L - 1].astype(np.float32)
    nrm2 = nrm2[:, : L - 1].astype(np.float32)

    norms = np.sqrt(nrm2)
    overlap = dots / (norms * np.sqrt(np.float32(K)) + np.float32(1e-8))
    return (np.float32(1.0) - overlap).astype(np.float32)
